# revision 7
# baseline (speedup 1.0000x reference)
"""Cross-modal attention (CMAttention) Trainium2 kernel.

Strategy: 8-way SPMD over (batch=4) x (modality=2). After the reference's
concat([q_x, q_a]) and 8-head split with head_dim=128, heads 0-3 depend only
on modality x and heads 4-7 only on modality a.  Each core therefore owns one
(batch, modality) pair end-to-end with zero communication.

Per-core pipeline (v2):
  stage A per 128-token tile: QKV projection (bf16 matmul, j-outer chains so
  the first chain only needs W[:,0:512]) + a 2-column mean chain (W column
  means precomputed on host) -> LN stats via Scalar Square+accum (sum q^2)
  and the matmul means -> LN apply (q on DVE tensor_scalar, k on Scalar ACT)
  -> RoPE (table multiplies on DVE) -> 8 per-head SBUF->SBUF DMA transposes
  (overlapped; no DRAM round trip, no serial transpose barrier).
  stage B: scores^T matmul -> exp on ScalarE (scale folded) -> attn @ [v | 1]
  (ones column yields the softmax denominator) -> normalize on DVE.
"""

import os
import sys

for _p in ("/opt/trn_rl_repo", os.path.expanduser("~/.axon_site/_ro/trn_rl_repo")):
    if os.path.isdir(_p) and _p not in sys.path:
        sys.path.append(_p)

from contextlib import ExitStack

import ml_dtypes
import numpy as np

import concourse.bacc as bacc
import concourse.bass as bass
import concourse.mybir as mybir
import concourse.tile as tile
from concourse.bass_utils import run_bass_kernel_spmd

BF16 = mybir.dt.float16
F32 = mybir.dt.float32
NPBF16 = np.float16

DIM = 512          # per-modality feature dim
N_TOK = 1024       # sequence length
NH = 4             # heads handled per core (one modality's heads)
D = 128            # head dim
NT = 8             # token tiles of 128
EPS = 1e-5
SCALE = 1.0 / float(np.sqrt(D))
VW = 132           # per-head v block width: 128 d + 1 ones + 3 pad


def build_module(trivial: bool):
    """Build the per-core Bass program.  trivial=True assumes all LN gains are
    exactly 1 and biases exactly 0 (folded tables are plain cos/sin and the
    additive rope term vanishes); trivial=False uses full-width tables with
    g folded in and an extra additive T3 table."""
    nc = bacc.Bacc("TRN2", target_bir_lowering=False, debug=False, num_devices=8)

    xT = nc.dram_tensor("xT", [DIM, N_TOK], BF16, kind="ExternalInput")
    W = nc.dram_tensor("W", [DIM, 3 * DIM], BF16, kind="ExternalInput")
    WMU = nc.dram_tensor("WMU", [DIM, 2], BF16, kind="ExternalInput")
    if trivial:
        T1 = nc.dram_tensor("T1", [N_TOK, 64], BF16, kind="ExternalInput")
        T2N = nc.dram_tensor("T2N", [N_TOK, 64], BF16, kind="ExternalInput")
        T2P = nc.dram_tensor("T2P", [N_TOK, 64], BF16, kind="ExternalInput")
    else:
        T1 = nc.dram_tensor("T1", [N_TOK, 1024], BF16, kind="ExternalInput")
        T2 = nc.dram_tensor("T2", [N_TOK, 1024], BF16, kind="ExternalInput")
        T3 = nc.dram_tensor("T3", [N_TOK, 1024], BF16, kind="ExternalInput")
    out_d = nc.dram_tensor("out", [N_TOK, DIM], F32, kind="ExternalOutput")

    with tile.TileContext(nc) as tc, ExitStack() as ctx:
        consts = ctx.enter_context(tc.tile_pool(name="consts", bufs=1))
        small = ctx.enter_context(tc.tile_pool(name="small", bufs=4))
        upool = ctx.enter_context(tc.tile_pool(name="upool", bufs=2))
        rpool = ctx.enter_context(tc.tile_pool(name="rpool", bufs=3))
        epool = ctx.enter_context(tc.tile_pool(name="epool", bufs=2))
        psum_qk = ctx.enter_context(tc.tile_pool(name="psqk", bufs=2, space="PSUM"))
        psum_v = ctx.enter_context(tc.tile_pool(name="psv", bufs=2, space="PSUM"))
        psum_mu = ctx.enter_context(tc.tile_pool(name="psmu", bufs=2, space="PSUM"))

        # ---- constants; ordered so the first qkv chain (t=0, j=0) is gated
        # only on the 4 xT t0-blocks + the 4 W[:,0:512] chunks ----
        xr = xT.ap().rearrange("(a b) c -> b a c", b=128)   # [128, 4, 1024]
        wr = W.ap().rearrange("(a b) c -> b a c", b=128)    # [128, 4, 1536]
        xT_k = [
            consts.tile([128, N_TOK], BF16, name=f"xT{kc}", tag=f"xT{kc}")
            for kc in range(4)
        ]
        W_k = [
            consts.tile([128, 3 * DIM], BF16, name=f"W{kc}", tag=f"W{kc}")
            for kc in range(4)
        ]
        for kc in range(4):  # t0 blocks of xT first
            nc.sync.dma_start(out=xT_k[kc][:, 0:128], in_=xr[:, kc, 0:128])
        for kc in range(4):  # q columns of W (first chains' rhs)
            eng = nc.sync if kc < 2 else nc.scalar
            eng.dma_start(out=W_k[kc][:, 0:512], in_=wr[:, kc, 0:512])
        wmu_sb = consts.tile([128, 4, 2], BF16, tag="wmu")
        nc.scalar.dma_start(
            out=wmu_sb, in_=WMU.ap().rearrange("(a b) c -> b a c", b=128)
        )
        for kc in range(4):  # rest of xT
            nc.sync.dma_start(out=xT_k[kc][:, 128:N_TOK], in_=xr[:, kc, 128:N_TOK])
        for kc in range(4):  # k columns of W
            nc.gpsimd.dma_start(out=W_k[kc][:, 512:1024], in_=wr[:, kc, 512:1024])
        for kc in range(4):  # v columns of W
            nc.gpsimd.dma_start(out=W_k[kc][:, 1024:1536], in_=wr[:, kc, 1024:1536])

        def _load_tiled(sbuf_tile, dram):
            nc.gpsimd.dma_start(
                out=sbuf_tile, in_=dram.ap().rearrange("(a b) c -> b a c", b=128)
            )

        if trivial:
            cos_sb = consts.tile([128, NT, 64], BF16, tag="cos")
            _load_tiled(cos_sb, T1)
            sinN_sb = consts.tile([128, NT, 64], BF16, tag="sinN")
            _load_tiled(sinN_sb, T2N)
            sinP_sb = consts.tile([128, NT, 64], BF16, tag="sinP")
            _load_tiled(sinP_sb, T2P)
        else:
            T1_sb = consts.tile([128, NT, 1024], BF16, tag="T1")
            _load_tiled(T1_sb, T1)
            T2_sb = consts.tile([128, NT, 1024], BF16, tag="T2")
            _load_tiled(T2_sb, T2)
            T3_sb = consts.tile([128, NT, 1024], BF16, tag="T3")
            _load_tiled(T3_sb, T3)
        eps_sb = consts.tile([128, 1], F32, tag="eps")
        nc.vector.memset(eps_sb, EPS)

        v_sb = consts.tile([128, NT, NH, VW], BF16, tag="v")
        nc.vector.memset(v_sb[:, :, :, 128:129], 1.0)

        qkT_sb = [
            [
                consts.tile(
                    [128, N_TOK], BF16, name=f"qkT{s}{h}", tag=f"qkT{s}{h}"
                )
                for h in range(NH)
            ]
            for s in range(2)
        ]
        out_sb = consts.tile([128, NT, DIM], F32, tag="osb")
        dpool = ctx.enter_context(tc.tile_pool(name="dpool", bufs=1, space="DRAM"))
        r_dram = dpool.tile([N_TOK, 2 * DIM], BF16, name="r_dram", tag="r_dram")

        def bcast(ap2d, dims):
            """[128, 64] AP -> [128, *dims, 64] with stride-0 broadcast dims."""
            p, last = ap2d.ap[0], ap2d.ap[-1]
            return bass.AP(
                tensor=ap2d.tensor,
                offset=ap2d.offset,
                ap=[p] + [[0, d] for d in dims] + [last],
            )

        def half(ap, i):
            return ap.rearrange("p (b half j) -> p b half j", half=2, j=64)[
                :, :, i, :
            ]

        # ---------------- stage A: one 128-token tile ----------------
        u_of = {}

        def stage_a1(t):
            tb = slice(t * 128, (t + 1) * 128)
            qkv_ps = psum_qk.tile([128, 2 * DIM], F32, tag="qk", name="qkv_ps")
            v_ps = psum_v.tile([128, DIM], F32, tag="v", name="v_ps")
            mu_ps = psum_mu.tile([128, 2], F32, tag="mu", name="mu_ps")
            for j in range(2):  # q then k chains
                for kc in range(4):
                    nc.tensor.matmul(
                        qkv_ps[:, j * 512 : (j + 1) * 512],
                        lhsT=xT_k[kc][:, tb],
                        rhs=W_k[kc][:, j * 512 : (j + 1) * 512],
                        start=(kc == 0),
                        stop=(kc == 3),
                    )
            for kc in range(4):  # v chain
                nc.tensor.matmul(
                    v_ps,
                    lhsT=xT_k[kc][:, tb],
                    rhs=W_k[kc][:, 1024:1536],
                    start=(kc == 0),
                    stop=(kc == 3),
                )
            for kc in range(4):  # mean chain: [mu_q, mu_k] per token row
                nc.tensor.matmul(
                    mu_ps,
                    lhsT=xT_k[kc][:, tb],
                    rhs=wmu_sb[:, kc],
                    start=(kc == 0),
                    stop=(kc == 3),
                )

            u = upool.tile([128, 2 * DIM], BF16, tag="u", name="u")
            mcp = small.tile([128, 2], F32, tag="mcp", name="mcp")
            nc.vector.tensor_copy(out=mcp, in_=mu_ps)
            for s in range(2):
                mu = mcp[:, s : s + 1]
                # sum of squares along features, on ScalarE (Square is in
                # every ACT table set, so no table thrash with Sqrt)
                sqj = small.tile([128, DIM], BF16, tag=f"sqj{s}", name="sqj")
                ss = small.tile([128, 1], F32, tag=f"ss{s}", name="ss")
                nc.scalar.activation(
                    out=sqj,
                    in_=qkv_ps[:, s * 512 : (s + 1) * 512],
                    func=mybir.ActivationFunctionType.Square,
                    accum_out=ss,
                )
                mu2 = small.tile([128, 1], F32, tag=f"mu2{s}", name="mu2")
                nc.vector.tensor_mul(mu2, mu, mu)
                var = small.tile([128, 1], F32, tag=f"var{s}", name="var")
                # var = ss/512 - mu^2
                nc.vector.scalar_tensor_tensor(
                    out=var,
                    in0=ss,
                    scalar=1.0 / 512.0,
                    in1=mu2,
                    op0=mybir.AluOpType.mult,
                    op1=mybir.AluOpType.subtract,
                )
                sd = small.tile([128, 1], F32, tag=f"sd{s}", name="sd")
                nc.scalar.activation(
                    sd, var, mybir.ActivationFunctionType.Sqrt, bias=eps_sb
                )
                rstd = small.tile([128, 1], F32, tag=f"rs{s}", name="rstd")
                nc.vector.reciprocal(rstd, sd)
                nmr = small.tile([128, 1], F32, tag=f"nmr{s}", name="nmr")
                nc.vector.scalar_tensor_tensor(
                    out=nmr,
                    in0=mu,
                    scalar=-1.0,
                    in1=rstd,
                    op0=mybir.AluOpType.mult,
                    op1=mybir.AluOpType.mult,
                )
                if s == 0:
                    # u = qkv*rstd + (-mu*rstd), on DVE
                    nc.vector.tensor_scalar(
                        out=u[:, 0:512],
                        in0=qkv_ps[:, 0:512],
                        scalar1=rstd,
                        scalar2=nmr,
                        op0=mybir.AluOpType.mult,
                        op1=mybir.AluOpType.add,
                    )
                else:
                    # same, on ScalarE (frees DVE)
                    nc.scalar.activation(
                        out=u[:, 512:1024],
                        in_=qkv_ps[:, 512:1024],
                        func=mybir.ActivationFunctionType.Identity,
                        scale=rstd,
                        bias=nmr,
                    )

            # v (raw) into augmented per-head layout, on ScalarE
            nc.scalar.activation(
                out=v_sb[:, t, :, 0:128],
                in_=v_ps.rearrange("p (h d) -> p h d", h=NH),
                func=mybir.ActivationFunctionType.Copy,
            )

            u_of[t] = u

        def stage_a2(t):
            u = u_of.pop(t)
            # rope: r = u * T1 + swap_half(u) * T2 (+ T3)
            m1 = rpool.tile([128, 2 * DIM], BF16, tag="m1", name="m1")
            m2 = rpool.tile([128, 2 * DIM], BF16, tag="m2", name="m2")
            r = rpool.tile([128, 2 * DIM], BF16, tag="r", name="r")
            if trivial:
                nc.vector.tensor_mul(
                    m1.rearrange("p (b j) -> p b j", j=64),
                    u.rearrange("p (b j) -> p b j", j=64),
                    bcast(cos_sb[:, t], (16,)),
                )
                nc.vector.tensor_mul(
                    half(m2, 0), half(u, 1), bcast(sinN_sb[:, t], (8,))
                )
                nc.vector.tensor_mul(
                    half(m2, 1), half(u, 0), bcast(sinP_sb[:, t], (8,))
                )
                nc.vector.tensor_add(r, m1, m2)
            else:
                t1v = T1_sb[:, t]
                t2v = T2_sb[:, t]
                t3v = T3_sb[:, t]
                nc.vector.tensor_mul(m1, u, t1v)
                nc.vector.tensor_mul(half(m2, 0), half(u, 1), half(t2v, 0))
                nc.vector.tensor_mul(half(m2, 1), half(u, 0), half(t2v, 1))
                nc.vector.tensor_add(m1, m1, m2)
                nc.vector.tensor_add(r, m1, t3v)
            # spill this tile's rope output on the (otherwise idle) SWDGE
            # ring; every tile pair, DMA-transpose [256,128] blocks back into
            # [d, tok] layout on the sync ring, overlapped with later tiles
            nc.gpsimd.dma_start(out=r_dram[t * 128 : (t + 1) * 128, :], in_=r)
            if t % 2 == 1:
                rows = slice((t - 1) * 128, (t + 1) * 128)
                for s in range(2):
                    for h in range(NH):
                        blk = (s * NH + h) * 128
                        nc.sync.dma_start(
                            out=qkT_sb[s][h][:, rows],
                            in_=r_dram[rows, blk : blk + 128],
                            transpose=True,
                        )

        # ---------------- stage B pieces ----------------
        # exp tiles: ets[h][kc] = exp(scores^T) [128 k, 1024 q] bf16
        ets = [[None] * NT for _ in range(NH)]

        def scores_kc(h, kc):
            qT, kT = qkT_sb[0][h], qkT_sb[1][h]
            sc_ps = psum_qk.tile([128, 2 * DIM], F32, tag="qk", name="sc_ps")
            for qh in range(2):
                nc.tensor.matmul(
                    sc_ps[:, qh * 512 : (qh + 1) * 512],
                    lhsT=kT[:, kc * 128 : (kc + 1) * 128],
                    rhs=qT[:, qh * 512 : (qh + 1) * 512],
                    start=True,
                    stop=True,
                )
            et = epool.tile(
                [128, N_TOK], BF16, tag=f"exp{h % 2}_{kc}",
                name=f"exp{h % 2}_{kc}", bufs=1,
            )
            nc.scalar.activation(
                out=et,
                in_=sc_ps[:, 0:N_TOK],
                func=mybir.ActivationFunctionType.Exp,
                scale=SCALE,
            )
            ets[h][kc] = et

        def emit_av_qc(h, qc):
            av = psum_v.tile([128, VW], F32, tag="v", name="av")
            for kc in range(NT):
                et = ets[h][kc]
                nc.tensor.matmul(
                    av[:, 0:129],
                    lhsT=et[:, qc * 128 : (qc + 1) * 128],
                    rhs=v_sb[:, kc, h, 0:129],
                    start=(kc == 0),
                    stop=(kc == NT - 1),
                )
            rcp = small.tile([128, 1], F32, tag="rcp", name="rcp")
            nc.vector.reciprocal(rcp, av[:, 128:129])
            dst = out_sb[:, qc, h * 128 : (h + 1) * 128]
            nc.vector.tensor_scalar_mul(dst, av[:, 0:128], rcp)
            if h == NH - 1:
                nc.gpsimd.dma_start(
                    out=out_d.ap()[qc * 128 : (qc + 1) * 128, :],
                    in_=out_sb[:, qc],
                )

        # ---------------- emission schedule ----------------
        stage_a1(0)
        for t in range(1, NT):
            stage_a1(t)
            stage_a2(t - 1)
        stage_a2(NT - 1)

        # warm the Exp ACT table during the A->B transition so the first
        # real exp doesn't pay the table switch
        warm = consts.tile([128, 1], F32, tag="warm")
        nc.scalar.activation(warm, eps_sb, mybir.ActivationFunctionType.Exp)

        for kc in range(NT):
            scores_kc(0, kc)
        for h in range(NH):
            for i in range(NT):
                if h + 1 < NH:
                    scores_kc(h + 1, i)
                emit_av_qc(h, i)

    nc.compile()
    return nc


def _rope_tables():
    inv_freq = 1.0 / (10000.0 ** (np.arange(0, D, 2, dtype=np.float32) / D))
    freqs = np.arange(N_TOK, dtype=np.float32)[:, None] * inv_freq[None, :]  # [n, 64]
    return np.cos(freqs), np.sin(freqs)


def _full_tables(g_q, b_q, g_k, b_k):
    """T1/T2/T3 [N_TOK, 1024] with LN gain/bias folded into the rope tables.
    Feature index layout matches u: (s, h, half, j)."""
    cos64, sin64 = _rope_tables()
    T1 = np.empty((N_TOK, 1024), np.float32)
    T2 = np.empty((N_TOK, 1024), np.float32)
    T3 = np.empty((N_TOK, 1024), np.float32)
    for s, (g, b) in enumerate(((g_q, b_q), (g_k, b_k))):
        g = g.reshape(NH, 2, 64)
        b = b.reshape(NH, 2, 64)
        for h in range(NH):
            base = s * 512 + h * 128
            lo, hi = slice(base, base + 64), slice(base + 64, base + 128)
            T1[:, lo] = g[h, 0] * cos64
            T1[:, hi] = g[h, 1] * cos64
            T2[:, lo] = -g[h, 1] * sin64
            T2[:, hi] = g[h, 0] * sin64
            T3[:, lo] = b[h, 0] * cos64 - b[h, 1] * sin64
            T3[:, hi] = b[h, 1] * cos64 + b[h, 0] * sin64
    return T1, T2, T3


def make_in_maps(x, a, Wqkv_x, Wqkv_a, g_qx, b_qx, g_kx, b_kx, g_qa, b_qa, g_ka, b_ka):
    """Returns (trivial, in_maps) for the 8 cores: core c = (batch c//2, modality c%2)."""
    x, a = np.asarray(x), np.asarray(a)
    Ws = (np.asarray(Wqkv_x), np.asarray(Wqkv_a))
    gb = (
        (np.asarray(g_qx), np.asarray(b_qx), np.asarray(g_kx), np.asarray(b_kx)),
        (np.asarray(g_qa), np.asarray(b_qa), np.asarray(g_ka), np.asarray(b_ka)),
    )
    trivial = all(
        np.all(g == 1.0) and np.all(b == 0.0)
        for (gq, bq, gk, bk) in gb
        for g, b in ((gq, bq), (gk, bk))
    )
    cos64, sin64 = _rope_tables()
    wmus = [
        np.stack([Wm[:, 0:512].mean(axis=1), Wm[:, 512:1024].mean(axis=1)], axis=1)
        for Wm in Ws
    ]
    in_maps = []
    for c in range(8):
        i, m = c // 2, c % 2
        src = x[i] if m == 0 else a[i]
        im = {
            "xT": np.ascontiguousarray(src.T).astype(NPBF16),
            "W": Ws[m].astype(NPBF16),
            "WMU": wmus[m].astype(NPBF16),
        }
        if trivial:
            im["T1"] = cos64.astype(NPBF16)
            im["T2N"] = (-sin64).astype(NPBF16)
            im["T2P"] = sin64.astype(NPBF16)
        else:
            gq, bq, gk, bk = gb[m]
            T1, T2, T3 = _full_tables(gq, bq, gk, bk)
            im["T1"] = T1.astype(NPBF16)
            im["T2"] = T2.astype(NPBF16)
            im["T3"] = T3.astype(NPBF16)
        in_maps.append(im)
    return trivial, in_maps


_module_cache: dict[bool, object] = {}


def _get_module(trivial: bool):
    if trivial not in _module_cache:
        _module_cache[trivial] = build_module(trivial)
    return _module_cache[trivial]


def kernel(**inputs) -> np.ndarray:
    trivial, in_maps = make_in_maps(**inputs)
    nc = _get_module(trivial)
    res = run_bass_kernel_spmd(nc, in_maps, core_ids=list(range(8)))
    out = np.empty((4, N_TOK, 2 * DIM), np.float32)
    for c in range(8):
        i, m = c // 2, c % 2
        out[i, :, m * 512 : (m + 1) * 512] = res.results[c]["out"]
    return out


# revision 11
# speedup vs baseline: 1.3625x; 1.3625x over previous
"""Cross-modal attention (CMAttention) Trainium2 kernel.

Strategy: 8-way SPMD over (batch=4) x (modality=2). After the reference's
concat([q_x, q_a]) and 8-head split with head_dim=128, heads 0-3 depend only
on modality x and heads 4-7 only on modality a.  Each core therefore owns one
(batch, modality) pair end-to-end with zero communication.

Per-core pipeline (v2):
  stage A per 128-token tile: QKV projection (bf16 matmul, j-outer chains so
  the first chain only needs W[:,0:512]) + a 2-column mean chain (W column
  means precomputed on host) -> LN stats via Scalar Square+accum (sum q^2)
  and the matmul means -> LN apply (q on DVE tensor_scalar, k on Scalar ACT)
  -> RoPE (table multiplies on DVE) -> 8 per-head SBUF->SBUF DMA transposes
  (overlapped; no DRAM round trip, no serial transpose barrier).
  stage B: scores^T matmul -> exp on ScalarE (scale folded) -> attn @ [v | 1]
  (ones column yields the softmax denominator) -> normalize on DVE.
"""

import os
import sys

for _p in ("/opt/trn_rl_repo", os.path.expanduser("~/.axon_site/_ro/trn_rl_repo")):
    if os.path.isdir(_p) and _p not in sys.path:
        sys.path.append(_p)

from contextlib import ExitStack

import ml_dtypes
import numpy as np

import concourse.bacc as bacc
import concourse.bass as bass
import concourse.mybir as mybir
import concourse.tile as tile
from concourse.bass_utils import run_bass_kernel_spmd

BF16 = mybir.dt.float16
F32 = mybir.dt.float32
NPBF16 = np.float16

DIM = 512          # per-modality feature dim
N_TOK = 1024       # sequence length
NH = 4             # heads handled per core (one modality's heads)
D = 128            # head dim
NT = 8             # token tiles of 128
EPS = 1e-5
SCALE = 1.0 / float(np.sqrt(D))
VW = 132           # per-head v block width: 128 d + 1 ones + 3 pad


def build_module(trivial: bool):
    """Build the per-core Bass program.  trivial=True assumes all LN gains are
    exactly 1 and biases exactly 0 (folded tables are plain cos/sin and the
    additive rope term vanishes); trivial=False uses full-width tables with
    g folded in and an extra additive T3 table."""
    nc = bacc.Bacc("TRN2", target_bir_lowering=False, debug=False, num_devices=8)

    xT = nc.dram_tensor("xT", [DIM, N_TOK], BF16, kind="ExternalInput")
    W = nc.dram_tensor("W", [DIM, 3 * DIM], BF16, kind="ExternalInput")
    WMU = nc.dram_tensor("WMU", [DIM, 2], BF16, kind="ExternalInput")
    if trivial:
        T1 = nc.dram_tensor("T1", [N_TOK, 64], BF16, kind="ExternalInput")
        T2N = nc.dram_tensor("T2N", [N_TOK, 64], BF16, kind="ExternalInput")
        T2P = nc.dram_tensor("T2P", [N_TOK, 64], BF16, kind="ExternalInput")
    else:
        T1 = nc.dram_tensor("T1", [N_TOK, 1024], BF16, kind="ExternalInput")
        T2 = nc.dram_tensor("T2", [N_TOK, 1024], BF16, kind="ExternalInput")
        T3 = nc.dram_tensor("T3", [N_TOK, 1024], BF16, kind="ExternalInput")
    out_d = nc.dram_tensor("out", [N_TOK, DIM], F32, kind="ExternalOutput")

    with tile.TileContext(nc) as tc, ExitStack() as ctx:
        consts = ctx.enter_context(tc.tile_pool(name="consts", bufs=1))
        small = ctx.enter_context(tc.tile_pool(name="small", bufs=4))
        upool = ctx.enter_context(tc.tile_pool(name="upool", bufs=2))
        rpool = ctx.enter_context(tc.tile_pool(name="rpool", bufs=3))
        epool = ctx.enter_context(tc.tile_pool(name="epool", bufs=2))
        psum_qk = ctx.enter_context(tc.tile_pool(name="psqk", bufs=2, space="PSUM"))
        psum_v = ctx.enter_context(tc.tile_pool(name="psv", bufs=2, space="PSUM"))
        psum_mu = ctx.enter_context(tc.tile_pool(name="psmu", bufs=2, space="PSUM"))

        # ---- constants; ordered so the first qkv chain (t=0, j=0) is gated
        # only on the 4 xT t0-blocks + the 4 W[:,0:512] chunks ----
        xr = xT.ap().rearrange("(a b) c -> b a c", b=128)   # [128, 4, 1024]
        wr = W.ap().rearrange("(a b) c -> b a c", b=128)    # [128, 4, 1536]
        xT_k = [
            consts.tile([128, N_TOK], BF16, name=f"xT{kc}", tag=f"xT{kc}")
            for kc in range(4)
        ]
        W_k = [
            consts.tile([128, 3 * DIM], BF16, name=f"W{kc}", tag=f"W{kc}")
            for kc in range(4)
        ]
        for kc in range(4):  # t0 blocks of xT first
            nc.sync.dma_start(out=xT_k[kc][:, 0:128], in_=xr[:, kc, 0:128])
        for kc in range(4):  # q columns of W (first chains' rhs)
            eng = nc.sync if kc < 2 else nc.scalar
            eng.dma_start(out=W_k[kc][:, 0:512], in_=wr[:, kc, 0:512])
        wmu_sb = consts.tile([128, 4, 2], BF16, tag="wmu")
        nc.scalar.dma_start(
            out=wmu_sb, in_=WMU.ap().rearrange("(a b) c -> b a c", b=128)
        )
        for kc in range(4):  # rest of xT
            nc.sync.dma_start(out=xT_k[kc][:, 128:N_TOK], in_=xr[:, kc, 128:N_TOK])
        for kc in range(4):  # k columns of W
            nc.gpsimd.dma_start(out=W_k[kc][:, 512:1024], in_=wr[:, kc, 512:1024])
        for kc in range(4):  # v columns of W
            nc.gpsimd.dma_start(out=W_k[kc][:, 1024:1536], in_=wr[:, kc, 1024:1536])

        def _load_tiled(sbuf_tile, dram):
            nc.gpsimd.dma_start(
                out=sbuf_tile, in_=dram.ap().rearrange("(a b) c -> b a c", b=128)
            )

        if trivial:
            cos_sb = consts.tile([128, NT, 64], BF16, tag="cos")
            _load_tiled(cos_sb, T1)
            sinN_sb = consts.tile([128, NT, 64], BF16, tag="sinN")
            _load_tiled(sinN_sb, T2N)
            sinP_sb = consts.tile([128, NT, 64], BF16, tag="sinP")
            _load_tiled(sinP_sb, T2P)
        else:
            T1_sb = consts.tile([128, NT, 1024], BF16, tag="T1")
            _load_tiled(T1_sb, T1)
            T2_sb = consts.tile([128, NT, 1024], BF16, tag="T2")
            _load_tiled(T2_sb, T2)
            T3_sb = consts.tile([128, NT, 1024], BF16, tag="T3")
            _load_tiled(T3_sb, T3)
        eps_sb = consts.tile([128, 1], F32, tag="eps")
        nc.vector.memset(eps_sb, EPS)

        v_sb = consts.tile([128, NT, NH, VW], BF16, tag="v")
        nc.vector.memset(v_sb[:, :, :, 128:129], 1.0)

        qkT_sb = [
            [
                consts.tile(
                    [128, N_TOK], BF16, name=f"qkT{s}{h}", tag=f"qkT{s}{h}"
                )
                for h in range(NH)
            ]
            for s in range(2)
        ]
        out_sb = consts.tile([128, NT, DIM], F32, tag="osb")
        dpool = ctx.enter_context(tc.tile_pool(name="dpool", bufs=1, space="DRAM"))
        r_dram = dpool.tile([N_TOK, 2 * DIM], BF16, name="r_dram", tag="r_dram")

        def bcast(ap2d, dims):
            """[128, 64] AP -> [128, *dims, 64] with stride-0 broadcast dims."""
            p, last = ap2d.ap[0], ap2d.ap[-1]
            return bass.AP(
                tensor=ap2d.tensor,
                offset=ap2d.offset,
                ap=[p] + [[0, d] for d in dims] + [last],
            )

        def half(ap, i):
            return ap.rearrange("p (b half j) -> p b half j", half=2, j=64)[
                :, :, i, :
            ]

        # ---------------- stage A: one 128-token tile ----------------
        u_of = {}

        def stage_a1(t):
            tb = slice(t * 128, (t + 1) * 128)
            qkv_ps = psum_qk.tile([128, 2 * DIM], F32, tag="qk", name="qkv_ps")
            v_ps = psum_v.tile([128, DIM], F32, tag="v", name="v_ps")
            mu_ps = psum_mu.tile([128, 2], F32, tag="mu", name="mu_ps")
            for j in range(2):  # q then k chains
                for kc in range(4):
                    nc.tensor.matmul(
                        qkv_ps[:, j * 512 : (j + 1) * 512],
                        lhsT=xT_k[kc][:, tb],
                        rhs=W_k[kc][:, j * 512 : (j + 1) * 512],
                        start=(kc == 0),
                        stop=(kc == 3),
                    )
            for kc in range(4):  # v chain
                nc.tensor.matmul(
                    v_ps,
                    lhsT=xT_k[kc][:, tb],
                    rhs=W_k[kc][:, 1024:1536],
                    start=(kc == 0),
                    stop=(kc == 3),
                )
            for kc in range(4):  # mean chain: [mu_q, mu_k] per token row
                nc.tensor.matmul(
                    mu_ps,
                    lhsT=xT_k[kc][:, tb],
                    rhs=wmu_sb[:, kc],
                    start=(kc == 0),
                    stop=(kc == 3),
                )

            u = upool.tile([128, 2 * DIM], BF16, tag="u", name="u")
            mcp = small.tile([128, 2], F32, tag="mcp", name="mcp")
            nc.vector.tensor_copy(out=mcp, in_=mu_ps)
            for s in range(2):
                mu = mcp[:, s : s + 1]
                # sum of squares along features, on ScalarE (Square is in
                # every ACT table set, so no table thrash with Sqrt)
                sqj = small.tile([128, DIM], BF16, tag=f"sqj{s}", name="sqj")
                ss = small.tile([128, 1], F32, tag=f"ss{s}", name="ss")
                nc.scalar.activation(
                    out=sqj,
                    in_=qkv_ps[:, s * 512 : (s + 1) * 512],
                    func=mybir.ActivationFunctionType.Square,
                    accum_out=ss,
                )
                mu2 = small.tile([128, 1], F32, tag=f"mu2{s}", name="mu2")
                nc.vector.tensor_mul(mu2, mu, mu)
                var = small.tile([128, 1], F32, tag=f"var{s}", name="var")
                # var = ss/512 - mu^2
                nc.vector.scalar_tensor_tensor(
                    out=var,
                    in0=ss,
                    scalar=1.0 / 512.0,
                    in1=mu2,
                    op0=mybir.AluOpType.mult,
                    op1=mybir.AluOpType.subtract,
                )
                sd = small.tile([128, 1], F32, tag=f"sd{s}", name="sd")
                nc.scalar.activation(
                    sd, var, mybir.ActivationFunctionType.Sqrt, bias=eps_sb
                )
                rstd = small.tile([128, 1], F32, tag=f"rs{s}", name="rstd")
                nc.vector.reciprocal(rstd, sd)
                nmr = small.tile([128, 1], F32, tag=f"nmr{s}", name="nmr")
                nc.vector.scalar_tensor_tensor(
                    out=nmr,
                    in0=mu,
                    scalar=-1.0,
                    in1=rstd,
                    op0=mybir.AluOpType.mult,
                    op1=mybir.AluOpType.mult,
                )
                if s == 0:
                    # u = qkv*rstd + (-mu*rstd), on DVE
                    nc.vector.tensor_scalar(
                        out=u[:, 0:512],
                        in0=qkv_ps[:, 0:512],
                        scalar1=rstd,
                        scalar2=nmr,
                        op0=mybir.AluOpType.mult,
                        op1=mybir.AluOpType.add,
                    )
                else:
                    # same, on ScalarE (frees DVE)
                    nc.scalar.activation(
                        out=u[:, 512:1024],
                        in_=qkv_ps[:, 512:1024],
                        func=mybir.ActivationFunctionType.Identity,
                        scale=rstd,
                        bias=nmr,
                    )

            # v (raw) into augmented per-head layout; alternate engines to
            # balance ScalarE/DVE load in stage A
            if t % 2 == 0:
                nc.scalar.activation(
                    out=v_sb[:, t, :, 0:128],
                    in_=v_ps.rearrange("p (h d) -> p h d", h=NH),
                    func=mybir.ActivationFunctionType.Copy,
                )
            else:
                nc.vector.tensor_copy(
                    out=v_sb[:, t, :, 0:128],
                    in_=v_ps.rearrange("p (h d) -> p h d", h=NH),
                )

            u_of[t] = u

        def stage_a2(t):
            u = u_of.pop(t)
            # rope: r = u * T1 + swap_half(u) * T2 (+ T3)
            m1 = rpool.tile([128, 2 * DIM], BF16, tag="m1", name="m1")
            m2 = rpool.tile([128, 2 * DIM], BF16, tag="m2", name="m2")
            r = rpool.tile([128, 2 * DIM], BF16, tag="r", name="r")
            if trivial:
                nc.vector.tensor_mul(
                    m1.rearrange("p (b j) -> p b j", j=64),
                    u.rearrange("p (b j) -> p b j", j=64),
                    bcast(cos_sb[:, t], (16,)),
                )
                nc.vector.tensor_mul(
                    half(m2, 0), half(u, 1), bcast(sinN_sb[:, t], (8,))
                )
                nc.vector.tensor_mul(
                    half(m2, 1), half(u, 0), bcast(sinP_sb[:, t], (8,))
                )
                nc.vector.tensor_add(r, m1, m2)
            else:
                t1v = T1_sb[:, t]
                t2v = T2_sb[:, t]
                t3v = T3_sb[:, t]
                nc.vector.tensor_mul(m1, u, t1v)
                nc.vector.tensor_mul(half(m2, 0), half(u, 1), half(t2v, 0))
                nc.vector.tensor_mul(half(m2, 1), half(u, 0), half(t2v, 1))
                nc.vector.tensor_add(m1, m1, m2)
                nc.vector.tensor_add(r, m1, t3v)
            # spill this tile's rope output on the (otherwise idle) SWDGE ring
            nc.gpsimd.dma_start(out=r_dram[t * 128 : (t + 1) * 128, :], in_=r)

        def transpose_head(h):
            # full-column transposes (DMA transpose cost is ~1.1us fixed per
            # instruction, so fewer/bigger is better); k before q so head h's
            # scores can start as soon as its pair lands
            for s in (1, 0):
                blk = (s * NH + h) * 128
                nc.sync.dma_start(
                    out=qkT_sb[s][h],
                    in_=r_dram[:, blk : blk + 128],
                    transpose=True,
                )

        # ---------------- stage B pieces ----------------
        # exp tiles: ets[h][kc] = exp(scores^T) [128 k, 1024 q] bf16
        ets = [[None] * NT for _ in range(NH)]

        def scores_kc(h, kc):
            qT, kT = qkT_sb[0][h], qkT_sb[1][h]
            sc_ps = psum_qk.tile([128, 2 * DIM], F32, tag="qk", name="sc_ps")
            for qh in range(2):
                nc.tensor.matmul(
                    sc_ps[:, qh * 512 : (qh + 1) * 512],
                    lhsT=kT[:, kc * 128 : (kc + 1) * 128],
                    rhs=qT[:, qh * 512 : (qh + 1) * 512],
                    start=True,
                    stop=True,
                )
            et = epool.tile(
                [128, N_TOK], BF16, tag=f"exp{h % 2}_{kc}",
                name=f"exp{h % 2}_{kc}", bufs=1,
            )
            nc.scalar.activation(
                out=et,
                in_=sc_ps[:, 0:N_TOK],
                func=mybir.ActivationFunctionType.Exp,
                scale=SCALE,
            )
            ets[h][kc] = et

        def emit_av_qc(h, qc):
            av = psum_v.tile([128, VW], F32, tag="v", name="av")
            for kc in range(NT):
                et = ets[h][kc]
                nc.tensor.matmul(
                    av[:, 0:129],
                    lhsT=et[:, qc * 128 : (qc + 1) * 128],
                    rhs=v_sb[:, kc, h, 0:129],
                    start=(kc == 0),
                    stop=(kc == NT - 1),
                )
            rcp = small.tile([128, 1], F32, tag="rcp", name="rcp")
            nc.vector.reciprocal(rcp, av[:, 128:129])
            dst = out_sb[:, qc, h * 128 : (h + 1) * 128]
            nc.vector.tensor_scalar_mul(dst, av[:, 0:128], rcp)
            if h == NH - 1:
                nc.gpsimd.dma_start(
                    out=out_d.ap()[qc * 128 : (qc + 1) * 128, :],
                    in_=out_sb[:, qc],
                )

        # ---------------- emission schedule ----------------
        stage_a1(0)
        for t in range(1, NT):
            stage_a1(t)
            stage_a2(t - 1)
        stage_a2(NT - 1)

        # warm the Exp ACT table during the A->B transition so the first
        # real exp doesn't pay the table switch
        warm = consts.tile([128, 1], F32, tag="warm")
        nc.scalar.activation(warm, eps_sb, mybir.ActivationFunctionType.Exp)

        for h in range(NH):
            transpose_head(h)
        for kc in range(NT):
            scores_kc(0, kc)
        for h in range(NH):
            for i in range(NT):
                if h + 1 < NH:
                    scores_kc(h + 1, i)
                emit_av_qc(h, i)

    nc.compile()
    return nc


def _rope_tables():
    inv_freq = 1.0 / (10000.0 ** (np.arange(0, D, 2, dtype=np.float32) / D))
    freqs = np.arange(N_TOK, dtype=np.float32)[:, None] * inv_freq[None, :]  # [n, 64]
    return np.cos(freqs), np.sin(freqs)


def _full_tables(g_q, b_q, g_k, b_k):
    """T1/T2/T3 [N_TOK, 1024] with LN gain/bias folded into the rope tables.
    Feature index layout matches u: (s, h, half, j)."""
    cos64, sin64 = _rope_tables()
    T1 = np.empty((N_TOK, 1024), np.float32)
    T2 = np.empty((N_TOK, 1024), np.float32)
    T3 = np.empty((N_TOK, 1024), np.float32)
    for s, (g, b) in enumerate(((g_q, b_q), (g_k, b_k))):
        g = g.reshape(NH, 2, 64)
        b = b.reshape(NH, 2, 64)
        for h in range(NH):
            base = s * 512 + h * 128
            lo, hi = slice(base, base + 64), slice(base + 64, base + 128)
            T1[:, lo] = g[h, 0] * cos64
            T1[:, hi] = g[h, 1] * cos64
            T2[:, lo] = -g[h, 1] * sin64
            T2[:, hi] = g[h, 0] * sin64
            T3[:, lo] = b[h, 0] * cos64 - b[h, 1] * sin64
            T3[:, hi] = b[h, 1] * cos64 + b[h, 0] * sin64
    return T1, T2, T3


def make_in_maps(x, a, Wqkv_x, Wqkv_a, g_qx, b_qx, g_kx, b_kx, g_qa, b_qa, g_ka, b_ka):
    """Returns (trivial, in_maps) for the 8 cores: core c = (batch c//2, modality c%2)."""
    x, a = np.asarray(x), np.asarray(a)
    Ws = (np.asarray(Wqkv_x), np.asarray(Wqkv_a))
    gb = (
        (np.asarray(g_qx), np.asarray(b_qx), np.asarray(g_kx), np.asarray(b_kx)),
        (np.asarray(g_qa), np.asarray(b_qa), np.asarray(g_ka), np.asarray(b_ka)),
    )
    trivial = all(
        np.all(g == 1.0) and np.all(b == 0.0)
        for (gq, bq, gk, bk) in gb
        for g, b in ((gq, bq), (gk, bk))
    )
    cos64, sin64 = _rope_tables()
    wmus = [
        np.stack([Wm[:, 0:512].mean(axis=1), Wm[:, 512:1024].mean(axis=1)], axis=1)
        for Wm in Ws
    ]
    in_maps = []
    for c in range(8):
        i, m = c // 2, c % 2
        src = x[i] if m == 0 else a[i]
        im = {
            "xT": np.ascontiguousarray(src.T).astype(NPBF16),
            "W": Ws[m].astype(NPBF16),
            "WMU": wmus[m].astype(NPBF16),
        }
        if trivial:
            im["T1"] = cos64.astype(NPBF16)
            im["T2N"] = (-sin64).astype(NPBF16)
            im["T2P"] = sin64.astype(NPBF16)
        else:
            gq, bq, gk, bk = gb[m]
            T1, T2, T3 = _full_tables(gq, bq, gk, bk)
            im["T1"] = T1.astype(NPBF16)
            im["T2"] = T2.astype(NPBF16)
            im["T3"] = T3.astype(NPBF16)
        in_maps.append(im)
    return trivial, in_maps


_module_cache: dict[bool, object] = {}


def _get_module(trivial: bool):
    if trivial not in _module_cache:
        _module_cache[trivial] = build_module(trivial)
    return _module_cache[trivial]


def kernel(**inputs) -> np.ndarray:
    trivial, in_maps = make_in_maps(**inputs)
    nc = _get_module(trivial)
    res = run_bass_kernel_spmd(nc, in_maps, core_ids=list(range(8)))
    out = np.empty((4, N_TOK, 2 * DIM), np.float32)
    for c in range(8):
        i, m = c // 2, c % 2
        out[i, :, m * 512 : (m + 1) * 512] = res.results[c]["out"]
    return out


# revision 18
# speedup vs baseline: 1.4589x; 1.0708x over previous
"""Cross-modal attention (CMAttention) Trainium2 kernel.

Strategy: 8-way SPMD over (batch=4) x (modality=2). After the reference's
concat([q_x, q_a]) and 8-head split with head_dim=128, heads 0-3 depend only
on modality x and heads 4-7 only on modality a.  Each core therefore owns one
(batch, modality) pair end-to-end with zero communication.

Per-core pipeline (v2):
  stage A per 128-token tile: QKV projection (bf16 matmul, j-outer chains so
  the first chain only needs W[:,0:512]) + a 2-column mean chain (W column
  means precomputed on host) -> LN stats via Scalar Square+accum (sum q^2)
  and the matmul means -> LN apply (q on DVE tensor_scalar, k on Scalar ACT)
  -> RoPE (table multiplies on DVE) -> 8 per-head SBUF->SBUF DMA transposes
  (overlapped; no DRAM round trip, no serial transpose barrier).
  stage B: scores^T matmul -> exp on ScalarE (scale folded) -> attn @ [v | 1]
  (ones column yields the softmax denominator) -> normalize on DVE.
"""

import os
import sys

for _p in ("/opt/trn_rl_repo", os.path.expanduser("~/.axon_site/_ro/trn_rl_repo")):
    if os.path.isdir(_p) and _p not in sys.path:
        sys.path.append(_p)

from contextlib import ExitStack

import ml_dtypes
import numpy as np

import concourse.bacc as bacc
import concourse.bass as bass
import concourse.mybir as mybir
import concourse.tile as tile
from concourse.bass_utils import run_bass_kernel_spmd

BF16 = mybir.dt.float16
F32 = mybir.dt.float32
NPBF16 = np.float16

DIM = 512          # per-modality feature dim
N_TOK = 1024       # sequence length
NH = 4             # heads handled per core (one modality's heads)
D = 128            # head dim
NT = 8             # token tiles of 128
EPS = 1e-5
SCALE = 1.0 / float(np.sqrt(D))
VW = 132           # per-head v block width: 128 d + 1 ones + 3 pad


def build_module(trivial: bool):
    """Build the per-core Bass program.  trivial=True assumes all LN gains are
    exactly 1 and biases exactly 0 (folded tables are plain cos/sin and the
    additive rope term vanishes); trivial=False uses full-width tables with
    g folded in and an extra additive T3 table."""
    nc = bacc.Bacc("TRN2", target_bir_lowering=False, debug=False, num_devices=8)

    xT = nc.dram_tensor("xT", [DIM, N_TOK], BF16, kind="ExternalInput")
    W = nc.dram_tensor("W", [DIM, 3 * DIM], BF16, kind="ExternalInput")
    WMU = nc.dram_tensor("WMU", [DIM, 2], BF16, kind="ExternalInput")
    if trivial:
        T1 = nc.dram_tensor("T1", [N_TOK, 64], BF16, kind="ExternalInput")
        T2N = nc.dram_tensor("T2N", [N_TOK, 64], BF16, kind="ExternalInput")
        T2P = nc.dram_tensor("T2P", [N_TOK, 64], BF16, kind="ExternalInput")
    else:
        T1 = nc.dram_tensor("T1", [N_TOK, 1024], BF16, kind="ExternalInput")
        T2 = nc.dram_tensor("T2", [N_TOK, 1024], BF16, kind="ExternalInput")
        T3 = nc.dram_tensor("T3", [N_TOK, 1024], BF16, kind="ExternalInput")
    out_d = nc.dram_tensor("out", [N_TOK, DIM], F32, kind="ExternalOutput")

    with tile.TileContext(nc) as tc, ExitStack() as ctx:
        consts = ctx.enter_context(tc.tile_pool(name="consts", bufs=1))
        small = ctx.enter_context(tc.tile_pool(name="small", bufs=4))
        upool = ctx.enter_context(tc.tile_pool(name="upool", bufs=2))
        rpool = ctx.enter_context(tc.tile_pool(name="rpool", bufs=3))
        epool = ctx.enter_context(tc.tile_pool(name="epool", bufs=2))
        psum_qk = ctx.enter_context(tc.tile_pool(name="psqk", bufs=2, space="PSUM"))
        psum_v = ctx.enter_context(tc.tile_pool(name="psv", bufs=2, space="PSUM"))
        psum_mu = ctx.enter_context(tc.tile_pool(name="psmu", bufs=2, space="PSUM"))

        # ---- constants; ordered so the first qkv chain (t=0, j=0) is gated
        # only on the 4 xT t0-blocks + the 4 W[:,0:512] chunks ----
        xr = xT.ap().rearrange("(a b) c -> b a c", b=128)   # [128, 4, 1024]
        wr = W.ap().rearrange("(a b) c -> b a c", b=128)    # [128, 4, 1536]
        # one writer per tile so tile-granular dependency tracking doesn't
        # serialize the first matmul chains behind unrelated loads
        xT_t0 = [
            consts.tile([128, 128], BF16, name=f"xT0_{kc}", tag=f"xT0_{kc}")
            for kc in range(4)
        ]
        xT_r = [
            consts.tile([128, N_TOK - 128], BF16, name=f"xTr{kc}", tag=f"xTr{kc}")
            for kc in range(4)
        ]
        W_q = [
            consts.tile([128, 512], BF16, name=f"Wq{kc}", tag=f"Wq{kc}")
            for kc in range(4)
        ]
        W_kk = [
            consts.tile([128, 512], BF16, name=f"Wk{kc}", tag=f"Wk{kc}")
            for kc in range(4)
        ]
        W_v = [
            consts.tile([128, 512], BF16, name=f"Wv{kc}", tag=f"Wv{kc}")
            for kc in range(4)
        ]

        def xblk(kc, t):
            if t == 0:
                return xT_t0[kc]
            return xT_r[kc][:, (t - 1) * 128 : t * 128]

        for kc in range(4):  # t0 blocks of xT first
            nc.sync.dma_start(out=xT_t0[kc], in_=xr[:, kc, 0:128])
        for kc in range(2):  # q columns of W (first chain's rhs)
            nc.sync.dma_start(out=W_q[kc], in_=wr[:, kc, 0:512])
        for kc in range(2, 4):
            nc.scalar.dma_start(out=W_q[kc], in_=wr[:, kc, 0:512])
        for kc in range(4):  # k columns of W
            nc.gpsimd.dma_start(out=W_kk[kc], in_=wr[:, kc, 512:1024])
        wmu_sb = consts.tile([128, 4, 2], BF16, tag="wmu")
        nc.scalar.dma_start(
            out=wmu_sb, in_=WMU.ap().rearrange("(a b) c -> b a c", b=128)
        )
        for kc in range(2):  # v columns of W
            nc.scalar.dma_start(out=W_v[kc], in_=wr[:, kc, 1024:1536])
        for kc in range(2, 4):
            nc.gpsimd.dma_start(out=W_v[kc], in_=wr[:, kc, 1024:1536])
        for kc in range(4):  # rest of xT
            nc.sync.dma_start(out=xT_r[kc], in_=xr[:, kc, 128:N_TOK])

        def _load_tiled(sbuf_tile, dram):
            nc.gpsimd.dma_start(
                out=sbuf_tile, in_=dram.ap().rearrange("(a b) c -> b a c", b=128)
            )

        if trivial:
            cos_sb = consts.tile([128, NT, 64], BF16, tag="cos")
            _load_tiled(cos_sb, T1)
            sinN_sb = consts.tile([128, NT, 64], BF16, tag="sinN")
            _load_tiled(sinN_sb, T2N)
            sinP_sb = consts.tile([128, NT, 64], BF16, tag="sinP")
            _load_tiled(sinP_sb, T2P)
        else:
            T1_sb = consts.tile([128, NT, 1024], BF16, tag="T1")
            _load_tiled(T1_sb, T1)
            T2_sb = consts.tile([128, NT, 1024], BF16, tag="T2")
            _load_tiled(T2_sb, T2)
            T3_sb = consts.tile([128, NT, 1024], BF16, tag="T3")
            _load_tiled(T3_sb, T3)
        eps_sb = consts.tile([128, 1], F32, tag="eps")
        nc.vector.memset(eps_sb, EPS)

        v_sb = consts.tile([128, NT, NH, VW], BF16, tag="v")
        nc.vector.memset(v_sb[:, :, :, 128:129], 1.0)

        qkT_sb = [
            [
                consts.tile(
                    [128, N_TOK], BF16, name=f"qkT{s}{h}", tag=f"qkT{s}{h}"
                )
                for h in range(NH)
            ]
            for s in range(2)
        ]
        out_sb = consts.tile([128, NT, DIM], F32, tag="osb")
        dpool = ctx.enter_context(tc.tile_pool(name="dpool", bufs=1, space="DRAM"))
        r_dram = dpool.tile([N_TOK, 2 * DIM], BF16, name="r_dram", tag="r_dram")

        def bcast(ap2d, dims):
            """[128, 64] AP -> [128, *dims, 64] with stride-0 broadcast dims."""
            p, last = ap2d.ap[0], ap2d.ap[-1]
            return bass.AP(
                tensor=ap2d.tensor,
                offset=ap2d.offset,
                ap=[p] + [[0, d] for d in dims] + [last],
            )

        def half(ap, i):
            return ap.rearrange("p (b half j) -> p b half j", half=2, j=64)[
                :, :, i, :
            ]

        # ---------------- stage A: one 128-token tile ----------------
        u_of = {}

        # rstd_k * SCALE per key tile, consumed by exp's per-partition scale
        rsk_sb = consts.tile([128, NT], F32, tag="rsk")

        def stage_a1(t):
            qkv_ps = psum_qk.tile([128, 2 * DIM], F32, tag="qk", name="qkv_ps")
            v_ps = psum_v.tile([128, DIM], F32, tag="v", name="v_ps")
            mu_ps = psum_mu.tile([128, 2], F32, tag="mu", name="mu_ps")
            for j in range(2):  # q then k chains
                W_j = W_q if j == 0 else W_kk
                for kc in range(4):
                    nc.tensor.matmul(
                        qkv_ps[:, j * 512 : (j + 1) * 512],
                        lhsT=xblk(kc, t),
                        rhs=W_j[kc],
                        start=(kc == 0),
                        stop=(kc == 3),
                    )
            for kc in range(4):  # mean chain: [mu_q, mu_k] per token row
                nc.tensor.matmul(
                    mu_ps,
                    lhsT=xblk(kc, t),
                    rhs=wmu_sb[:, kc],
                    start=(kc == 0),
                    stop=(kc == 3),
                )
            for kc in range(4):  # v chain
                nc.tensor.matmul(
                    v_ps,
                    lhsT=xblk(kc, t),
                    rhs=W_v[kc],
                    start=(kc == 0),
                    stop=(kc == 3),
                )

            # LN is decoupled: u = q - mu frees the PSUM quickly; the rstd
            # scale is exact to defer (rope is linear per token): rstd_q is
            # applied post-rope, rstd_k*SCALE goes into exp's ACT scale.
            u = upool.tile([128, 2 * DIM], BF16, tag="u", name="u")
            mcp = small.tile([128, 2], F32, tag="mcp", name="mcp")
            nc.vector.tensor_copy(out=mcp, in_=mu_ps)
            negmk = small.tile([128, 1], F32, tag="negmk", name="negmk")
            nc.vector.tensor_scalar_mul(negmk, mcp[:, 1:2], -1.0)
            nc.vector.tensor_scalar_sub(u[:, 0:512], qkv_ps[:, 0:512], mcp[:, 0:1])
            nc.scalar.activation(
                out=u[:, 512:1024],
                in_=qkv_ps[:, 512:1024],
                func=mybir.ActivationFunctionType.Identity,
                bias=negmk,
            )
            rstd_q = rstd_k = None
            for s in range(2):
                mu = mcp[:, s : s + 1]
                # sum of squares along features, on ScalarE (Square is in
                # every ACT table set, so no table thrash with Sqrt)
                sqj = small.tile([128, DIM], BF16, tag=f"sqj{s}", name="sqj")
                ss = small.tile([128, 1], F32, tag=f"ss{s}", name="ss")
                nc.scalar.activation(
                    out=sqj,
                    in_=qkv_ps[:, s * 512 : (s + 1) * 512],
                    func=mybir.ActivationFunctionType.Square,
                    accum_out=ss,
                )
                mu2 = small.tile([128, 1], F32, tag=f"mu2{s}", name="mu2")
                nc.vector.tensor_mul(mu2, mu, mu)
                var = small.tile([128, 1], F32, tag=f"var{s}", name="var")
                # var = ss/512 - mu^2
                nc.vector.scalar_tensor_tensor(
                    out=var,
                    in0=ss,
                    scalar=1.0 / 512.0,
                    in1=mu2,
                    op0=mybir.AluOpType.mult,
                    op1=mybir.AluOpType.subtract,
                )
                sd = small.tile([128, 1], F32, tag=f"sd{s}", name="sd")
                nc.scalar.activation(
                    sd, var, mybir.ActivationFunctionType.Sqrt, bias=eps_sb
                )
                rstd = small.tile([128, 1], F32, tag=f"rs{s}", name="rstd")
                nc.vector.reciprocal(rstd, sd)
                if s == 0:
                    rstd_q = rstd
                elif trivial:
                    nc.vector.tensor_scalar_mul(
                        rsk_sb[:, t : t + 1], rstd, SCALE
                    )
                else:
                    rstd_k = rstd

            # v (raw) into augmented per-head layout; alternate engines to
            # balance ScalarE/DVE load in stage A
            if t % 2 == 0:
                nc.scalar.activation(
                    out=v_sb[:, t, :, 0:128],
                    in_=v_ps.rearrange("p (h d) -> p h d", h=NH),
                    func=mybir.ActivationFunctionType.Copy,
                )
            else:
                nc.vector.tensor_copy(
                    out=v_sb[:, t, :, 0:128],
                    in_=v_ps.rearrange("p (h d) -> p h d", h=NH),
                )

            u_of[t] = (u, rstd_q, rstd_k)

        def stage_a2(t):
            u, rstd_q, rstd_k = u_of.pop(t)
            rows = slice(t * 128, (t + 1) * 128)
            # rope on un-normalized u: r = u * T1 + swap_half(u) * T2 (+ T3)
            m1 = rpool.tile([128, 2 * DIM], BF16, tag="m1", name="m1")
            m2 = rpool.tile([128, 2 * DIM], BF16, tag="m2", name="m2")
            r = rpool.tile([128, 2 * DIM], BF16, tag="r", name="r")
            if trivial:
                nc.vector.tensor_mul(
                    m1.rearrange("p (b j) -> p b j", j=64),
                    u.rearrange("p (b j) -> p b j", j=64),
                    bcast(cos_sb[:, t], (16,)),
                )
                nc.vector.tensor_mul(
                    half(m2, 0), half(u, 1), bcast(sinN_sb[:, t], (8,))
                )
                nc.vector.tensor_mul(
                    half(m2, 1), half(u, 0), bcast(sinP_sb[:, t], (8,))
                )
                nc.vector.tensor_add(r, m1, m2)
                # deferred LN scale: q rows get rstd_q here; k rows get
                # rstd_k (*SCALE) inside the exp activation instead
                rq = rpool.tile([128, DIM], BF16, tag="rq", name="rq")
                nc.vector.tensor_scalar_mul(rq, r[:, 0:512], rstd_q)
                nc.gpsimd.dma_start(out=r_dram[rows, 0:512], in_=rq)
                nc.gpsimd.dma_start(out=r_dram[rows, 512:1024], in_=r[:, 512:1024])
            else:
                t1v = T1_sb[:, t]
                t2v = T2_sb[:, t]
                t3v = T3_sb[:, t]
                nc.vector.tensor_mul(m1, u, t1v)
                nc.vector.tensor_mul(half(m2, 0), half(u, 1), half(t2v, 0))
                nc.vector.tensor_mul(half(m2, 1), half(u, 0), half(t2v, 1))
                nc.vector.tensor_add(m1, m1, m2)
                # r = (m1+m2)*rstd + T3, per LN group
                for s, rstd in ((0, rstd_q), (1, rstd_k)):
                    cols = slice(s * 512, (s + 1) * 512)
                    nc.vector.scalar_tensor_tensor(
                        out=r[:, cols],
                        in0=m1[:, cols],
                        scalar=rstd,
                        in1=t3v[:, cols],
                        op0=mybir.AluOpType.mult,
                        op1=mybir.AluOpType.add,
                    )
                nc.gpsimd.dma_start(out=r_dram[rows, :], in_=r)

        def transpose_head(h):
            # full-column transposes (DMA transpose cost is ~1.1us fixed per
            # instruction, so fewer/bigger is better); k before q so head h's
            # scores can start as soon as its pair lands
            for s in (1, 0):
                blk = (s * NH + h) * 128
                nc.sync.dma_start(
                    out=qkT_sb[s][h],
                    in_=r_dram[:, blk : blk + 128],
                    transpose=True,
                )

        # ---------------- stage B pieces ----------------
        # exp tiles: ets[h][kc] = exp(scores^T) [128 k, 1024 q] bf16
        ets = [[None] * NT for _ in range(NH)]

        def scores_kc(h, kc):
            qT, kT = qkT_sb[0][h], qkT_sb[1][h]
            sc_ps = psum_qk.tile([128, 2 * DIM], F32, tag="qk", name="sc_ps")
            for qh in range(2):
                nc.tensor.matmul(
                    sc_ps[:, qh * 512 : (qh + 1) * 512],
                    lhsT=kT[:, kc * 128 : (kc + 1) * 128],
                    rhs=qT[:, qh * 512 : (qh + 1) * 512],
                    start=True,
                    stop=True,
                )
            et = epool.tile(
                [128, N_TOK], BF16, tag=f"exp{h % 2}_{kc}",
                name=f"exp{h % 2}_{kc}", bufs=1,
            )
            nc.scalar.activation(
                out=et,
                in_=sc_ps[:, 0:N_TOK],
                func=mybir.ActivationFunctionType.Exp,
                scale=rsk_sb[:, kc : kc + 1] if trivial else SCALE,
            )
            ets[h][kc] = et

        def emit_av_qc(h, qc):
            av = psum_v.tile([128, VW], F32, tag="v", name="av")
            for kc in range(NT):
                et = ets[h][kc]
                nc.tensor.matmul(
                    av[:, 0:129],
                    lhsT=et[:, qc * 128 : (qc + 1) * 128],
                    rhs=v_sb[:, kc, h, 0:129],
                    start=(kc == 0),
                    stop=(kc == NT - 1),
                )
            rcp = small.tile([128, 1], F32, tag="rcp", name="rcp")
            nc.vector.reciprocal(rcp, av[:, 128:129])
            dst = out_sb[:, qc, h * 128 : (h + 1) * 128]
            nc.vector.tensor_scalar_mul(dst, av[:, 0:128], rcp)
            if h == NH - 1:
                nc.gpsimd.dma_start(
                    out=out_d.ap()[qc * 128 : (qc + 1) * 128, :],
                    in_=out_sb[:, qc],
                )

        # ---------------- emission schedule ----------------
        stage_a1(0)
        for t in range(1, NT):
            stage_a1(t)
            stage_a2(t - 1)
        stage_a2(NT - 1)

        # warm the Exp ACT table during the A->B transition so the first
        # real exp doesn't pay the table switch
        warm = consts.tile([128, 1], F32, tag="warm")
        nc.scalar.activation(warm, eps_sb, mybir.ActivationFunctionType.Exp)

        for h in range(NH):
            transpose_head(h)
        for kc in range(NT):
            scores_kc(0, kc)
        for h in range(NH):
            for i in range(NT):
                if h + 1 < NH:
                    scores_kc(h + 1, i)
                emit_av_qc(h, i)

    nc.compile()
    return nc


def _rope_tables():
    inv_freq = 1.0 / (10000.0 ** (np.arange(0, D, 2, dtype=np.float32) / D))
    freqs = np.arange(N_TOK, dtype=np.float32)[:, None] * inv_freq[None, :]  # [n, 64]
    return np.cos(freqs), np.sin(freqs)


def _full_tables(g_q, b_q, g_k, b_k):
    """T1/T2/T3 [N_TOK, 1024] with LN gain/bias folded into the rope tables.
    Feature index layout matches u: (s, h, half, j)."""
    cos64, sin64 = _rope_tables()
    T1 = np.empty((N_TOK, 1024), np.float32)
    T2 = np.empty((N_TOK, 1024), np.float32)
    T3 = np.empty((N_TOK, 1024), np.float32)
    for s, (g, b) in enumerate(((g_q, b_q), (g_k, b_k))):
        g = g.reshape(NH, 2, 64)
        b = b.reshape(NH, 2, 64)
        for h in range(NH):
            base = s * 512 + h * 128
            lo, hi = slice(base, base + 64), slice(base + 64, base + 128)
            T1[:, lo] = g[h, 0] * cos64
            T1[:, hi] = g[h, 1] * cos64
            T2[:, lo] = -g[h, 1] * sin64
            T2[:, hi] = g[h, 0] * sin64
            T3[:, lo] = b[h, 0] * cos64 - b[h, 1] * sin64
            T3[:, hi] = b[h, 1] * cos64 + b[h, 0] * sin64
    return T1, T2, T3


def make_in_maps(x, a, Wqkv_x, Wqkv_a, g_qx, b_qx, g_kx, b_kx, g_qa, b_qa, g_ka, b_ka):
    """Returns (trivial, in_maps) for the 8 cores: core c = (batch c//2, modality c%2)."""
    x, a = np.asarray(x), np.asarray(a)
    Ws = (np.asarray(Wqkv_x), np.asarray(Wqkv_a))
    gb = (
        (np.asarray(g_qx), np.asarray(b_qx), np.asarray(g_kx), np.asarray(b_kx)),
        (np.asarray(g_qa), np.asarray(b_qa), np.asarray(g_ka), np.asarray(b_ka)),
    )
    trivial = all(
        np.all(g == 1.0) and np.all(b == 0.0)
        for (gq, bq, gk, bk) in gb
        for g, b in ((gq, bq), (gk, bk))
    )
    cos64, sin64 = _rope_tables()
    wmus = [
        np.stack([Wm[:, 0:512].mean(axis=1), Wm[:, 512:1024].mean(axis=1)], axis=1)
        for Wm in Ws
    ]
    in_maps = []
    for c in range(8):
        i, m = c // 2, c % 2
        src = x[i] if m == 0 else a[i]
        im = {
            "xT": np.ascontiguousarray(src.T).astype(NPBF16),
            "W": Ws[m].astype(NPBF16),
            "WMU": wmus[m].astype(NPBF16),
        }
        if trivial:
            im["T1"] = cos64.astype(NPBF16)
            im["T2N"] = (-sin64).astype(NPBF16)
            im["T2P"] = sin64.astype(NPBF16)
        else:
            gq, bq, gk, bk = gb[m]
            T1, T2, T3 = _full_tables(gq, bq, gk, bk)
            im["T1"] = T1.astype(NPBF16)
            im["T2"] = T2.astype(NPBF16)
            im["T3"] = T3.astype(NPBF16)
        in_maps.append(im)
    return trivial, in_maps


_module_cache: dict[bool, object] = {}


def _get_module(trivial: bool):
    if trivial not in _module_cache:
        _module_cache[trivial] = build_module(trivial)
    return _module_cache[trivial]


def kernel(**inputs) -> np.ndarray:
    trivial, in_maps = make_in_maps(**inputs)
    nc = _get_module(trivial)
    res = run_bass_kernel_spmd(nc, in_maps, core_ids=list(range(8)))
    out = np.empty((4, N_TOK, 2 * DIM), np.float32)
    for c in range(8):
        i, m = c // 2, c % 2
        out[i, :, m * 512 : (m + 1) * 512] = res.results[c]["out"]
    return out
